# revision 8
# baseline (speedup 1.0000x reference)
"""MultiHeadEMA (MEGA bidirectional EMA + residual + SiLU) on 8 Trainium2 cores.

Strategy
--------
Per channel d (E=1024, B=4, L=4096):
    y[n] = silu( sum_{m<=n} x[m] k1[d, n-m] + sum_{m>n} x[m] k2[d, m-n-1]
                 + omega[d] x[n] )
with k1/k2 16-term geometric mixtures, q = 1 - sigmoid(a)*sigmoid(d) <= 0.865.
q^32 tail: worst-channel L1 1.4e-2 << 2e-2 * scale(16.9), so the length-2L FFT
conv reduces to a +-T=32-tap banded conv done by overlap-save with DFT F=256,
hop C=192 (22 windows).  E sharded 8 ways (128 ch/core, FREE = B*128 = 512).

Matmul cost on TRN2 is free-dim cycles (~216 ns at N=512 fp16) plus ~100 ns
whenever a partial-partition matmul breaks LDWEIGHTS pipelining, so every
matmul is full 128x128 config: x is staged TWICE (aligned + shifted by 64
rows) so odd windows contract 2 aligned tiles of the shifted copy, and the
half-block inverse ranges use zero-padded V columns.  174 matmuls total
(88 fwd + 86 inv) vs 300 for the F=512/T=64 baseline.

Freq packing (256-pt real DFT -> 256 real rows in 2 PSUM banks):
    X0 rows f=0..127:  Re X[f]
    X1 row 0: Re X[128] (Nyquist); rows 1..127: Im X[f]
Pointwise complex multiply via 4 coefficient planes:
    Y0 = A0*X0 + B0*X1 ; Y1 = A1*X1 + B1*X0
omega residual folded into tap 0 host-side.

Windows are processed in PAIRS: the two forward DFTs write adjacent PSUM
banks and all elementwise ops run once per pair at FD=1024.  The pair's X
banks are EVACUATED first (DVE copies X0, ACT copies X1, ~1.2 us) so the
single-buffered PSUM tiles free quickly and the next pair's matmuls never
stall; the multiplies then run in SBUF fp16 at DVE 2x mode (~594 ns/pair).
DVE and GpSimd share an SBUF port (contention measured ~+40%), so GpSimd
gets only t10 (plus t11 on odd pairs).  Inverse matmuls for the pair are
emitted y0-chunks-first so PE work is available while the GpSimd-dependent
y1 finishes; split output blocks accumulate two windows' inverse matmuls
into one PSUM bank (second window start=False on zero-padded V columns).
"""

import math
import numpy as np
from contextlib import ExitStack

import concourse.bass as bass
import concourse.tile as tile
from concourse import bacc, mybir
from concourse.bass_utils import run_bass_kernel_spmd

L, B, E, NDIM = 4096, 4, 1024, 16
N_CORES = 8
ESH = E // N_CORES            # 128 channels per core
F, T, C = 256, 32, 192        # DFT length, one-sided taps, hop
NW = (L + C - 1) // C         # 22 windows (last covers 64 outputs)
NP = (NW + 1) // 2            # 11 window pairs
FREE = B * ESH                # 512 free elements (b, chan)
NXT = 33                      # x tiles: rows [0, 4224), x at [T, T+L)
NBLK = L // 128               # 32 output blocks

F16 = mybir.dt.float16
F32 = mybir.dt.float32

LAST_RESULTS = None           # BassKernelResults of the most recent run
_CACHE: dict = {}


def _build_nc():
    nc = bacc.Bacc("TRN2", target_bir_lowering=False, debug=False,
                   num_devices=N_CORES)
    xs = nc.dram_tensor("xs", [NXT * 128, B, ESH], F16, kind="ExternalInput").ap()
    xh_ = nc.dram_tensor("xsh", [NXT * 128, B, ESH], F16, kind="ExternalInput").ap()
    wf = nc.dram_tensor("wf", [128, 2, 2, 128], F16, kind="ExternalInput").ap()
    vi = nc.dram_tensor("vi", [128, 2, 4, 128], F16, kind="ExternalInput").ap()
    kco = nc.dram_tensor("kco", [128, 4, 2, FREE], F16, kind="ExternalInput").ap()
    out = nc.dram_tensor("out", [L, B, ESH], F16, kind="ExternalOutput").ap()

    with ExitStack() as ctx:
        tc = ctx.enter_context(tile.TileContext(nc))
        cpool = ctx.enter_context(tc.tile_pool(name="const", bufs=1))
        ppool = ctx.enter_context(tc.tile_pool(name="pw", bufs=2))
        opool = ctx.enter_context(tc.tile_pool(name="outp", bufs=3))
        ps_f = ctx.enter_context(tc.tile_pool(name="psf", bufs=1, space="PSUM"))
        ps_i = ctx.enter_context(tc.tile_pool(name="psi", bufs=1, space="PSUM"))

        # DMA order = first-use order across Sync / Scalar / GpSimd queues.
        wf_t = cpool.tile([128, 2, 2, 128], F16)
        nc.scalar.dma_start(wf_t[:], wf)
        x_all = cpool.tile([128, NXT, FREE], F16)
        xr = xs.rearrange("(t p) b c -> p t (b c)", p=128)
        nc.sync.dma_start(x_all[:, 0:4, :], xr[:, 0:4, :])
        xsh_all = cpool.tile([128, NXT, FREE], F16)
        xshr = xh_.rearrange("(t p) b c -> p t (b c)", p=128)
        nc.gpsimd.dma_start(xsh_all[:, 0:4, :], xshr[:, 0:4, :])
        k_t = cpool.tile([128, 4, 2, FREE], F16)
        nc.scalar.dma_start(k_t[:], kco)
        vi_t = cpool.tile([128, 2, 4, 128], F16)
        nc.scalar.dma_start(vi_t[:], vi)
        for t0 in range(4, NXT, 6):
            t1 = min(t0 + 6, NXT)
            nc.sync.dma_start(x_all[:, t0:t1, :], xr[:, t0:t1, :])
            nc.gpsimd.dma_start(xsh_all[:, t0:t1, :], xshr[:, t0:t1, :])

        def fwd(w, xh):
            """forward 256-pt real DFT of window w -> bank w%2 of X0/X1 pair"""
            if w % 2 == 0:
                src, a = x_all, 3 * w // 2
            else:
                src, a = xsh_all, (3 * w - 1) // 2
            for b in range(2):
                for k in range(2):
                    nc.tensor.matmul(xh[b][:, w % 2, :], wf_t[:, b, k, :],
                                     src[:, a + k, :],
                                     start=(k == 0), stop=(k == 1))

        blk_tiles = {}

        def get_blk(bi):
            if bi not in blk_tiles:
                blk_tiles[bi] = ps_i.tile([128, FREE], F32, tag=f"yi{bi % 3}",
                                          name=f"yi{bi}")
            return blk_tiles[bi]

        def silu_store(bi):
            yi = blk_tiles.pop(bi)
            o_sb = opool.tile([128, FREE], F16, tag=f"o{bi % 3}", name=f"o{bi}")
            nc.scalar.activation(o_sb[:], yi[:],
                                 mybir.ActivationFunctionType.Silu)
            nc.sync.dma_start(
                out[128 * bi: 128 * (bi + 1), :, :].rearrange("p b c -> p (b c)"),
                o_sb[:])

        def pointwise(p, xh):
            """FD=1024 elementwise over the window pair (2p, 2p+1).
            Evacuate PSUM first so the X banks free early, then multiply in
            SBUF where DVE runs 2x mode."""
            X0, X1 = xh
            x0s = ppool.tile([128, 2, FREE], F16, tag="x0s", name=f"x0s_{p}")
            nc.vector.tensor_copy(x0s[:], X0[:])
            x1s = ppool.tile([128, 2, FREE], F16, tag="x1s", name=f"x1s_{p}")
            nc.scalar.copy(x1s[:], X1[:])
            t00 = ppool.tile([128, 2, FREE], F16, tag="t00", name=f"t00_{p}")
            nc.vector.tensor_mul(t00[:], x0s[:], k_t[:, 0, :, :])
            t01 = ppool.tile([128, 2, FREE], F16, tag="t01", name=f"t01_{p}")
            nc.vector.tensor_mul(t01[:], x1s[:], k_t[:, 1, :, :])
            y0 = ppool.tile([128, 2, FREE], F16, tag="y0", name=f"y0_{p}")
            nc.vector.tensor_add(y0[:], t00[:], t01[:])
            t10 = ppool.tile([128, 2, FREE], F16, tag="t10", name=f"t10_{p}")
            nc.gpsimd.tensor_mul(t10[:], x1s[:], k_t[:, 2, :, :])
            t11 = ppool.tile([128, 2, FREE], F16, tag="t11", name=f"t11_{p}")
            if p % 2 == 0:
                nc.vector.tensor_mul(t11[:], x0s[:], k_t[:, 3, :, :])
            else:
                nc.gpsimd.tensor_mul(t11[:], x0s[:], k_t[:, 3, :, :])
            y1 = ppool.tile([128, 2, FREE], F16, tag="y1", name=f"y1_{p}")
            nc.vector.tensor_add(y1[:], t10[:], t11[:])
            return y0, y1

        def inv_ranges(w):
            # (block, vseg, first_writer, window)
            if w % 2 == 0:
                rr = [(3 * w // 2, 0, True, w), (3 * w // 2 + 1, 1, True, w)]
            else:
                rr = [((3 * w - 1) // 2, 3, False, w),
                      ((3 * w + 1) // 2, 2, True, w)]
            return [r for r in rr if r[0] < NBLK]

        def inv_pair(p, y0, y1):
            ranges = inv_ranges(2 * p)
            if 2 * p + 1 < NW:
                ranges += inv_ranges(2 * p + 1)
            # y0 chunks first (ready early), then y1 chunks (GpSimd path)
            for bi, seg, first, w in ranges:
                nc.tensor.matmul(get_blk(bi)[:], vi_t[:, 0, seg, :],
                                 y0[:, w % 2, :], start=first, stop=False,
                                 skip_group_check=True)
            for bi, seg, first, w in ranges:
                nc.tensor.matmul(get_blk(bi)[:], vi_t[:, 1, seg, :],
                                 y1[:, w % 2, :], start=False, stop=True,
                                 skip_group_check=True)
            for bi in (3 * p, 3 * p + 1, 3 * p + 2):
                if bi < NBLK:
                    silu_store(bi)

        # PE pre-warm: dummy matmuls keep the HAM activity monitor busy while
        # the first x tiles stream in, so real matmuls start near 2.4 GHz.
        warm = ps_i.tile([128, FREE], F32, tag="yi0", name="warm")
        for r in range(12):
            nc.tensor.matmul(warm[:, 0:256], wf_t[:, 0, 0, :],
                             wf_t[:, 0, :, :], start=(r == 0), stop=(r == 11))

        def fwd_pair(p, xh):
            fwd(2 * p, xh)
            if 2 * p + 1 < NW:
                fwd(2 * p + 1, xh)

        def x_tiles(p):
            return [ps_f.tile([128, 2, FREE], F32, tag=f"x{b}",
                              name=f"x{b}_{p}") for b in range(2)]

        # pipeline: evacuate pair p, then queue fwd(p+1) (waits only on the
        # evacuation), then pair p's SBUF pointwise + inverse.
        xh_cur = x_tiles(0)
        fwd_pair(0, xh_cur)
        for p in range(NP):
            y0, y1 = pointwise(p, xh_cur)
            if p + 1 < NP:
                xh_next = x_tiles(p + 1)
                fwd_pair(p + 1, xh_next)
            else:
                xh_next = None
            inv_pair(p, y0, y1)
            xh_cur = xh_next
    nc.compile()
    return nc


def _host_prep(x, alpha, delta, beta, gamma, omega):
    """Fold EMA params into freq-domain coefficient planes + DFT matrices."""
    a = 1.0 / (1.0 + np.exp(-alpha.astype(np.float64)))
    d = 1.0 / (1.0 + np.exp(-delta.astype(np.float64)))
    q = 1.0 - a * d                               # (2E, 16, 1)
    w = (a * beta.astype(np.float64))[:, :, 0] * gamma.astype(np.float64)
    w *= math.sqrt(1.0 / NDIM)                    # (2E, 16)
    tau = np.arange(128)
    kern = (w[:, :, None] * q[:, :, 0:1] ** tau[None, None, :]).sum(1)  # (2E,128)
    k1, k2 = kern[:E], kern[E:]
    kc = np.zeros((E, F))
    kc[:, 0:128] = k1
    kc[:, F - 127:] = k2[:, :127][:, ::-1]        # slot F-i holds k2[i-1]
    kc[:, 0] += omega.astype(np.float64)          # residual == omega on tap 0
    Khat = np.fft.rfft(kc, axis=1)                # (E, 129)
    KRe, KIm = Khat.real, Khat.imag

    planes = np.zeros((4, 128, E))                # A0, B0, A1, B1
    planes[0] = KRe[:, 0:128].T
    planes[1, 1:] = -KIm[:, 1:128].T
    planes[2, 0] = KRe[:, 128]
    planes[2, 1:] = KRe[:, 1:128].T
    planes[3, 1:] = KIm[:, 1:128].T

    # forward DFT lhsT packs: W0 (Re rows), W1 (Nyquist + Im rows) [256, 128]
    t_ = np.arange(F)
    fr = np.arange(128)
    W0 = np.cos(2 * np.pi * np.outer(t_, fr) / F)
    W1 = np.empty((F, 128))
    W1[:, 0] = np.cos(np.pi * t_)
    W1[:, 1:] = -np.sin(2 * np.pi * np.outer(t_, fr[1:]) / F)
    Wb = np.stack([W0, W1], axis=0)               # (2, 256, 128)
    wf = np.empty((128, 2, 2, 128))               # chunk k = rows 128k+p
    for k in range(2):
        wf[:, :, k, :] = Wb[:, 128 * k:128 * (k + 1), :].transpose(1, 0, 2)

    # inverse lhsT: V0/V1 [128 freq, 192 outs], zero-padded per range
    jj = np.arange(C) + T
    c_f = np.where(fr == 0, 1.0, 2.0)
    V0 = c_f[:, None] * np.cos(2 * np.pi * np.outer(fr, jj) / F) / F
    V1 = np.empty((128, C))
    V1[0] = ((-1.0) ** jj) / F
    V1[1:] = -2 * np.sin(2 * np.pi * np.outer(fr[1:], jj) / F) / F
    Vb = np.stack([V0, V1], axis=0)               # (2, 128, 192)
    vi = np.zeros((128, 2, 4, 128))
    vi[:, :, 0, :] = Vb[:, :, 0:128].transpose(1, 0, 2)      # even range A
    vi[:, :, 1, 0:64] = Vb[:, :, 128:192].transpose(1, 0, 2)  # even range B
    vi[:, :, 2, :] = Vb[:, :, 64:192].transpose(1, 0, 2)     # odd range B
    vi[:, :, 3, 64:128] = Vb[:, :, 0:64].transpose(1, 0, 2)  # odd range A

    xpad = np.zeros((NXT * 128, B, E), np.float16)
    xpad[T:T + L] = x.astype(np.float16)
    xsh = np.zeros((NXT * 128, B, E), np.float16)
    xsh[:NXT * 128 - 64] = xpad[64:]              # shifted copy for odd windows

    wf16 = np.ascontiguousarray(wf.astype(np.float16))
    vi16 = np.ascontiguousarray(vi.astype(np.float16))
    in_maps = []
    for core in range(N_CORES):
        sl = slice(core * ESH, (core + 1) * ESH)
        kc1 = np.broadcast_to(
            planes.reshape(4, 128, 1, 1, E)[:, :, :, :, sl],
            (4, 128, 2, B, ESH)).reshape(4, 128, 2, FREE).transpose(1, 0, 2, 3)
        in_maps.append({
            "xs": np.ascontiguousarray(xpad[:, :, sl]),
            "xsh": np.ascontiguousarray(xsh[:, :, sl]),
            "wf": wf16,
            "vi": vi16,
            "kco": np.ascontiguousarray(kc1.astype(np.float16)),
        })
    return in_maps


def kernel(x, alpha, delta, beta, gamma, omega):
    global LAST_RESULTS
    if "nc" not in _CACHE:
        _CACHE["nc"] = _build_nc()
    nc = _CACHE["nc"]
    in_maps = _host_prep(x, alpha, delta, beta, gamma, omega)
    res = run_bass_kernel_spmd(nc, in_maps, core_ids=list(range(N_CORES)))
    LAST_RESULTS = res
    out = np.concatenate([res.results[c]["out"] for c in range(N_CORES)], axis=2)
    return out.astype(np.float32)


# revision 10
# speedup vs baseline: 1.3384x; 1.3384x over previous
"""MultiHeadEMA (MEGA bidirectional EMA + residual + SiLU) on 8 Trainium2 cores.

Strategy
--------
Per channel d (E=1024, B=4, L=4096):
    y[n] = silu( sum_{m<=n} x[m] k1[d, n-m] + sum_{m>n} x[m] k2[d, m-n-1]
                 + omega[d] x[n] )
with k1/k2 16-term geometric mixtures, q = 1 - sigmoid(a)*sigmoid(d) <= 0.865.
q^32 tail: worst-channel L1 1.4e-2 << 2e-2 * scale(16.9), so the length-2L FFT
conv reduces to a +-T=32-tap banded conv done by overlap-save with DFT F=256,
hop C=192 (22 windows).  E sharded 8 ways (128 ch/core, FREE = B*128 = 512).

Matmul cost on TRN2 is free-dim cycles (~216 ns at N=512 fp16) plus ~100 ns
whenever a partial-partition matmul breaks LDWEIGHTS pipelining, so every
matmul is full 128x128 config: x is staged TWICE (aligned + shifted 64 rows)
so odd windows contract 2 aligned tiles of the shifted copy, and half-block
inverse ranges use zero-padded V columns.  174 matmuls (88 fwd + 86 inv).

Freq packing (256-pt real DFT -> 256 real rows in 2 PSUM banks):
    X0 rows f=0..127:  Re X[f]
    X1 row 0: Re X[128] (Nyquist); rows 1..127: Im X[f]
Pointwise complex multiply, Y0 = A0*X0 + B0*X1 ; Y1 = A1*X1 + B1*X0, with
the omega residual folded into tap 0 host-side.

Engine law on TRN2 (measured): GpSimd tensor ops and DVE 2-port copy/cast
modes serialize on a shared SBUF port (exclusive lock), so GpSimd does NO
compute here and all PSUM evacuation runs on ScalarE.  Windows run in PAIRS:
    ACT : copy X0 pair + X1 pair PSUM -> one SBUF tile x01s (2x 1.15 us),
          one fused SiLU over the pair's 3 finished output banks (1.6 us)
    DVE : m0 = x01s*[A0|B0], m1 = x01s*[B1|A1] (FD=2048, 2x mode, 1.13 us
          each), y0/y1 = half + half adds (FD=1024, 0.6 us each)
    PE  : fwd 8 + inv 8 matmuls per pair (~220 ns each)
    GPS : only SWDGE issue of the shifted-x DMAs
Inverse matmuls are emitted y0-chunks-first; the pair's 3 output blocks
live in one 3-bank PSUM tile (split block accumulates both windows,
second window start=False), SiLU reads all 3 at once and one DMA stores
384 output rows.
"""

import math
import numpy as np
from contextlib import ExitStack

import concourse.bass as bass
import concourse.tile as tile
from concourse import bacc, mybir
from concourse.bass_utils import run_bass_kernel_spmd

L, B, E, NDIM = 4096, 4, 1024, 16
N_CORES = 8
ESH = E // N_CORES            # 128 channels per core
F, T, C = 256, 32, 192        # DFT length, one-sided taps, hop
NW = (L + C - 1) // C         # 22 windows (last covers 64 outputs)
NP = (NW + 1) // 2            # 11 window pairs
FREE = B * ESH                # 512 free elements (b, chan)
NXT = 33                      # x tiles: rows [0, 4224), x at [T, T+L)
NBLK = L // 128               # 32 output blocks

F16 = mybir.dt.float16
F32 = mybir.dt.float32

LAST_RESULTS = None           # BassKernelResults of the most recent run
_CACHE: dict = {}


def _build_nc():
    nc = bacc.Bacc("TRN2", target_bir_lowering=False, debug=False,
                   num_devices=N_CORES)
    xs = nc.dram_tensor("xs", [NXT * 128, B, ESH], F16, kind="ExternalInput").ap()
    xh_ = nc.dram_tensor("xsh", [NXT * 128, B, ESH], F16, kind="ExternalInput").ap()
    wf = nc.dram_tensor("wf", [128, 2, 2, 128], F16, kind="ExternalInput").ap()
    vi = nc.dram_tensor("vi", [128, 2, 4, 128], F16, kind="ExternalInput").ap()
    kco = nc.dram_tensor("kco", [128, 2, 2, 2, FREE], F16,
                         kind="ExternalInput").ap()
    out = nc.dram_tensor("out", [L, B, ESH], F16, kind="ExternalOutput").ap()

    with ExitStack() as ctx:
        tc = ctx.enter_context(tile.TileContext(nc))
        cpool = ctx.enter_context(tc.tile_pool(name="const", bufs=1))
        ppool = ctx.enter_context(tc.tile_pool(name="pw", bufs=2))
        opool = ctx.enter_context(tc.tile_pool(name="outp", bufs=2))
        ps_f = ctx.enter_context(tc.tile_pool(name="psf", bufs=1, space="PSUM"))
        ps_i = ctx.enter_context(tc.tile_pool(name="psi", bufs=1, space="PSUM"))

        # DMA: consts + aligned x on Sync/Scalar HWDGE queues, shifted x via
        # GpSimd SWDGE (its only job).  Order = first-use order.
        wf_t = cpool.tile([128, 2, 2, 128], F16)
        nc.scalar.dma_start(wf_t[:], wf)
        x_all = cpool.tile([128, NXT, FREE], F16)
        xr = xs.rearrange("(t p) b c -> p t (b c)", p=128)
        nc.sync.dma_start(x_all[:, 0:4, :], xr[:, 0:4, :])
        xsh_all = cpool.tile([128, NXT, FREE], F16)
        xshr = xh_.rearrange("(t p) b c -> p t (b c)", p=128)
        nc.gpsimd.dma_start(xsh_all[:, 0:4, :], xshr[:, 0:4, :])
        k_t = cpool.tile([128, 2, 2, 2, FREE], F16)
        nc.scalar.dma_start(k_t[:], kco)
        vi_t = cpool.tile([128, 2, 4, 128], F16)
        nc.scalar.dma_start(vi_t[:], vi)
        for t0 in range(4, NXT, 6):
            t1 = min(t0 + 6, NXT)
            nc.sync.dma_start(x_all[:, t0:t1, :], xr[:, t0:t1, :])
            nc.gpsimd.dma_start(xsh_all[:, t0:t1, :], xshr[:, t0:t1, :])

        def fwd(w, xh):
            """forward 256-pt real DFT of window w -> bank w%2 of X0/X1 pair"""
            if w % 2 == 0:
                src, a = x_all, 3 * w // 2
            else:
                src, a = xsh_all, (3 * w - 1) // 2
            for b in range(2):
                for k in range(2):
                    nc.tensor.matmul(xh[b][:, w % 2, :], wf_t[:, b, k, :],
                                     src[:, a + k, :],
                                     start=(k == 0), stop=(k == 1))

        def pointwise(p, xh):
            """pair-fused elementwise: evacuate on ACT, multiply/add on DVE"""
            X0, X1 = xh
            x01 = ppool.tile([128, 2, 2, FREE], F16, tag="x01", name=f"x01_{p}")
            nc.scalar.copy(x01[:, 0, :, :], X0[:])
            nc.scalar.copy(x01[:, 1, :, :], X1[:])
            m0 = ppool.tile([128, 2, 2, FREE], F16, tag="m0", name=f"m0_{p}")
            nc.vector.tensor_mul(m0[:], x01[:], k_t[:, 0, :, :, :])
            y0 = ppool.tile([128, 2, FREE], F16, tag="y0", name=f"y0_{p}")
            nc.vector.tensor_add(y0[:], m0[:, 0, :, :], m0[:, 1, :, :])
            m1 = ppool.tile([128, 2, 2, FREE], F16, tag="m1", name=f"m1_{p}")
            nc.vector.tensor_mul(m1[:], x01[:], k_t[:, 1, :, :, :])
            y1 = ppool.tile([128, 2, FREE], F16, tag="y1", name=f"y1_{p}")
            nc.vector.tensor_add(y1[:], m1[:, 0, :, :], m1[:, 1, :, :])
            return y0, y1

        def inv_ranges(w):
            # (pair-slice, vseg, first_writer, window)
            if w % 2 == 0:
                return [(0, 0, True, w), (1, 1, True, w)]
            return [(1, 3, False, w), (2, 2, True, w)]

        def inv_pair(p, y0, y1):
            """inverse DFT for the pair into one 3-bank PSUM tile, then one
            fused SiLU + one DMA for output rows [384p, 384p+384)."""
            nb = min(3, NBLK - 3 * p)
            yi = ps_i.tile([128, 3, FREE], F32, tag="yi", name=f"yi{p}")
            ranges = inv_ranges(2 * p)
            if 2 * p + 1 < NW:
                ranges += inv_ranges(2 * p + 1)
            ranges = [r for r in ranges if r[0] < nb]
            for sl, seg, first, w in ranges:
                nc.tensor.matmul(yi[:, sl, :], vi_t[:, 0, seg, :],
                                 y0[:, w % 2, :], start=first, stop=False,
                                 skip_group_check=True)
            for sl, seg, first, w in ranges:
                nc.tensor.matmul(yi[:, sl, :], vi_t[:, 1, seg, :],
                                 y1[:, w % 2, :], start=False, stop=True,
                                 skip_group_check=True)
            o_sb = opool.tile([128, 3, FREE], F16, tag=f"o{p % 2}", name=f"o{p}")
            nc.scalar.activation(o_sb[:, 0:nb, :], yi[:, 0:nb, :],
                                 mybir.ActivationFunctionType.Silu)
            nc.sync.dma_start(
                out[384 * p: 384 * p + 128 * nb, :, :]
                .rearrange("(t p) b c -> p t (b c)", p=128),
                o_sb[:, 0:nb, :])

        # PE pre-warm: dummy matmuls keep the HAM activity monitor busy while
        # the first x tiles stream in, so real matmuls start near 2.4 GHz.
        warm = ps_i.tile([128, 3, FREE], F32, tag="yi", name="warm")
        for r in range(12):
            nc.tensor.matmul(warm[:, 0, 0:256], wf_t[:, 0, 0, :],
                             wf_t[:, 0, :, :], start=(r == 0), stop=(r == 11))

        def fwd_pair(p, xh):
            fwd(2 * p, xh)
            if 2 * p + 1 < NW:
                fwd(2 * p + 1, xh)

        def x_tiles(p):
            return [ps_f.tile([128, 2, FREE], F32, tag=f"x{b}",
                              name=f"x{b}_{p}") for b in range(2)]

        # pipeline: evacuate pair p (ACT), queue fwd(p+1) (waits only on the
        # evacuation), then pair p's DVE pointwise + inverse.
        xh_cur = x_tiles(0)
        fwd_pair(0, xh_cur)
        for p in range(NP):
            y0, y1 = pointwise(p, xh_cur)
            if p + 1 < NP:
                xh_next = x_tiles(p + 1)
                fwd_pair(p + 1, xh_next)
            else:
                xh_next = None
            inv_pair(p, y0, y1)
            xh_cur = xh_next
    nc.compile()
    return nc


def _host_prep(x, alpha, delta, beta, gamma, omega):
    """Fold EMA params into freq-domain coefficient planes + DFT matrices."""
    a = 1.0 / (1.0 + np.exp(-alpha.astype(np.float64)))
    d = 1.0 / (1.0 + np.exp(-delta.astype(np.float64)))
    q = 1.0 - a * d                               # (2E, 16, 1)
    w = (a * beta.astype(np.float64))[:, :, 0] * gamma.astype(np.float64)
    w *= math.sqrt(1.0 / NDIM)                    # (2E, 16)
    tau = np.arange(128)
    kern = (w[:, :, None] * q[:, :, 0:1] ** tau[None, None, :]).sum(1)  # (2E,128)
    k1, k2 = kern[:E], kern[E:]
    kc = np.zeros((E, F))
    kc[:, 0:128] = k1
    kc[:, F - 127:] = k2[:, :127][:, ::-1]        # slot F-i holds k2[i-1]
    kc[:, 0] += omega.astype(np.float64)          # residual == omega on tap 0
    Khat = np.fft.rfft(kc, axis=1)                # (E, 129)
    KRe, KIm = Khat.real, Khat.imag

    planes = np.zeros((4, 128, E))                # A0, B0, A1, B1
    planes[0] = KRe[:, 0:128].T
    planes[1, 1:] = -KIm[:, 1:128].T
    planes[2, 0] = KRe[:, 128]
    planes[2, 1:] = KRe[:, 1:128].T
    planes[3, 1:] = KIm[:, 1:128].T

    # forward DFT lhsT packs: W0 (Re rows), W1 (Nyquist + Im rows) [256, 128]
    t_ = np.arange(F)
    fr = np.arange(128)
    W0 = np.cos(2 * np.pi * np.outer(t_, fr) / F)
    W1 = np.empty((F, 128))
    W1[:, 0] = np.cos(np.pi * t_)
    W1[:, 1:] = -np.sin(2 * np.pi * np.outer(t_, fr[1:]) / F)
    Wb = np.stack([W0, W1], axis=0)               # (2, 256, 128)
    wf = np.empty((128, 2, 2, 128))               # chunk k = rows 128k+p
    for k in range(2):
        wf[:, :, k, :] = Wb[:, 128 * k:128 * (k + 1), :].transpose(1, 0, 2)

    # inverse lhsT: V0/V1 [128 freq, 192 outs], zero-padded per range
    jj = np.arange(C) + T
    c_f = np.where(fr == 0, 1.0, 2.0)
    V0 = c_f[:, None] * np.cos(2 * np.pi * np.outer(fr, jj) / F) / F
    V1 = np.empty((128, C))
    V1[0] = ((-1.0) ** jj) / F
    V1[1:] = -2 * np.sin(2 * np.pi * np.outer(fr[1:], jj) / F) / F
    Vb = np.stack([V0, V1], axis=0)               # (2, 128, 192)
    vi = np.zeros((128, 2, 4, 128))
    vi[:, :, 0, :] = Vb[:, :, 0:128].transpose(1, 0, 2)      # even range A
    vi[:, :, 1, 0:64] = Vb[:, :, 128:192].transpose(1, 0, 2)  # even range B
    vi[:, :, 2, :] = Vb[:, :, 64:192].transpose(1, 0, 2)     # odd range B
    vi[:, :, 3, 64:128] = Vb[:, :, 0:64].transpose(1, 0, 2)  # odd range A

    xpad = np.zeros((NXT * 128, B, E), np.float16)
    xpad[T:T + L] = x.astype(np.float16)
    xsh = np.zeros((NXT * 128, B, E), np.float16)
    xsh[:NXT * 128 - 64] = xpad[64:]              # shifted copy for odd windows

    wf16 = np.ascontiguousarray(wf.astype(np.float16))
    vi16 = np.ascontiguousarray(vi.astype(np.float16))
    # fused-mul plane packs: m0 halves [A0 | B0], m1 halves [B1 | A1],
    # each repeated over the window pair axis.
    pk = np.stack([np.stack([planes[0], planes[1]]),
                   np.stack([planes[3], planes[2]])])   # (2, 2, 128, E)
    in_maps = []
    for core in range(N_CORES):
        sl = slice(core * ESH, (core + 1) * ESH)
        kc1 = np.broadcast_to(
            pk.reshape(2, 2, 128, 1, 1, E)[:, :, :, :, :, sl],
            (2, 2, 128, 2, B, ESH)).reshape(2, 2, 128, 2, FREE)
        kc1 = kc1.transpose(2, 0, 1, 3, 4)        # (128, m, half, pair, FREE)
        in_maps.append({
            "xs": np.ascontiguousarray(xpad[:, :, sl]),
            "xsh": np.ascontiguousarray(xsh[:, :, sl]),
            "wf": wf16,
            "vi": vi16,
            "kco": np.ascontiguousarray(kc1.astype(np.float16)),
        })
    return in_maps


def kernel(x, alpha, delta, beta, gamma, omega):
    global LAST_RESULTS
    if "nc" not in _CACHE:
        _CACHE["nc"] = _build_nc()
    nc = _CACHE["nc"]
    in_maps = _host_prep(x, alpha, delta, beta, gamma, omega)
    res = run_bass_kernel_spmd(nc, in_maps, core_ids=list(range(N_CORES)))
    LAST_RESULTS = res
    out = np.concatenate([res.results[c]["out"] for c in range(N_CORES)], axis=2)
    return out.astype(np.float32)


# revision 14
# speedup vs baseline: 1.3616x; 1.0173x over previous
"""MultiHeadEMA (MEGA bidirectional EMA + residual + SiLU) on 8 Trainium2 cores.

Strategy
--------
Per channel d (E=1024, B=4, L=4096):
    y[n] = silu( sum_{m<=n} x[m] k1[d, n-m] + sum_{m>n} x[m] k2[d, m-n-1]
                 + omega[d] x[n] )
with k1/k2 16-term geometric mixtures, q = 1 - sigmoid(a)*sigmoid(d) <= 0.865.
q^32 tail: worst-channel L1 1.4e-2 << 2e-2 * scale(16.9), so the length-2L FFT
conv reduces to a +-T=32-tap banded conv done by overlap-save with DFT F=256,
hop C=192 (22 windows).  E sharded 8 ways (128 ch/core, FREE = B*128 = 512).

Matmul cost on TRN2 is free-dim cycles (~216 ns at N=512 fp16) plus ~100 ns
whenever a partial-partition matmul breaks LDWEIGHTS pipelining, so every
matmul is full 128x128 config: x is staged TWICE (aligned + shifted 64 rows)
so odd windows contract 2 aligned tiles of the shifted copy, and half-block
inverse ranges use zero-padded V columns.  174 matmuls (88 fwd + 86 inv).

Freq packing (256-pt real DFT -> 256 real rows in 2 PSUM banks):
    X0 rows f=0..127:  Re X[f]
    X1 row 0: Re X[128] (Nyquist); rows 1..127: Im X[f]
Pointwise complex multiply, Y0 = A0*X0 + B0*X1 ; Y1 = A1*X1 + B1*X0, with
the omega residual folded into tap 0 host-side.

Engine law on TRN2 (measured): GpSimd tensor ops and DVE 2-port copy/cast
modes serialize on a shared SBUF port (exclusive lock), so GpSimd does NO
compute here and all PSUM evacuation runs on ScalarE.  Windows run in PAIRS:
    ACT : copy X0 pair + X1 pair PSUM -> one SBUF tile x01s (2x 1.15 us),
          one fused SiLU over the pair's 3 finished output banks (1.6 us)
    DVE : m0 = x01s*[A0|B0], m1 = x01s*[B1|A1] (FD=2048, 2x mode, 1.13 us
          each), y0/y1 = half + half adds (FD=1024, 0.6 us each)
    PE  : fwd 8 + inv 8 matmuls per pair (~220 ns each)
    GPS : only SWDGE issue of the shifted-x DMAs
Inverse matmuls are emitted y0-chunks-first; the pair's 3 output blocks
live in one 3-bank PSUM tile (split block accumulates both windows,
second window start=False), SiLU reads all 3 at once and one DMA stores
384 output rows.
"""

import math
import numpy as np
from contextlib import ExitStack

import concourse.bass as bass
import concourse.tile as tile
from concourse import bacc, mybir
from concourse.bass_utils import run_bass_kernel_spmd

L, B, E, NDIM = 4096, 4, 1024, 16
N_CORES = 8
ESH = E // N_CORES            # 128 channels per core
F, T, C = 256, 32, 192        # DFT length, one-sided taps, hop
NW = (L + C - 1) // C         # 22 windows (last covers 64 outputs)
NP = (NW + 1) // 2            # 11 window pairs
FREE = B * ESH                # 512 free elements (b, chan)
NXT = 33                      # x tiles: rows [0, 4224), x at [T, T+L)
NBLK = L // 128               # 32 output blocks

F16 = mybir.dt.float16
F32 = mybir.dt.float32

LAST_RESULTS = None           # BassKernelResults of the most recent run
_CACHE: dict = {}


def _build_nc():
    nc = bacc.Bacc("TRN2", target_bir_lowering=False, debug=False,
                   num_devices=N_CORES)
    xs = nc.dram_tensor("xs", [NXT * 128, B, ESH], F16, kind="ExternalInput").ap()
    xh_ = nc.dram_tensor("xsh", [NXT * 128, B, ESH], F16, kind="ExternalInput").ap()
    wf = nc.dram_tensor("wf", [128, 2, 2, 128], F16, kind="ExternalInput").ap()
    vi = nc.dram_tensor("vi", [128, 2, 4, 128], F16, kind="ExternalInput").ap()
    kco = nc.dram_tensor("kco", [128, 2, 2, 2, FREE], F16,
                         kind="ExternalInput").ap()
    out = nc.dram_tensor("out", [L, B, ESH], F16, kind="ExternalOutput").ap()

    with ExitStack() as ctx:
        tc = ctx.enter_context(tile.TileContext(nc))
        cpool = ctx.enter_context(tc.tile_pool(name="const", bufs=1))
        ppool = ctx.enter_context(tc.tile_pool(name="pw", bufs=2))
        opool = ctx.enter_context(tc.tile_pool(name="outp", bufs=2))
        ps_f = ctx.enter_context(tc.tile_pool(name="psf", bufs=1, space="PSUM"))
        ps_i = ctx.enter_context(tc.tile_pool(name="psi", bufs=1, space="PSUM"))

        # DMA: consts + aligned x on Sync/Scalar HWDGE queues, shifted x via
        # GpSimd SWDGE (its only job).  Order = first-use order.
        wf_t = cpool.tile([128, 2, 2, 128], F16)
        nc.scalar.dma_start(wf_t[:], wf)
        x_all = cpool.tile([128, NXT, FREE], F16)
        xr = xs.rearrange("(t p) b c -> p t (b c)", p=128)
        nc.sync.dma_start(x_all[:, 0:6, :], xr[:, 0:6, :])
        xsh_all = cpool.tile([128, NXT, FREE], F16)
        xshr = xh_.rearrange("(t p) b c -> p t (b c)", p=128)
        nc.gpsimd.dma_start(xsh_all[:, 0:6, :], xshr[:, 0:6, :])
        k_t = cpool.tile([128, 2, 2, 2, FREE], F16)
        nc.scalar.dma_start(k_t[:], kco)
        vi_t = cpool.tile([128, 2, 4, 128], F16)
        nc.scalar.dma_start(vi_t[:], vi)
        for t0 in range(6, NXT, 6):
            t1 = min(t0 + 6, NXT)
            nc.sync.dma_start(x_all[:, t0:t1, :], xr[:, t0:t1, :])
            nc.gpsimd.dma_start(xsh_all[:, t0:t1, :], xshr[:, t0:t1, :])

        def fwd(w, xh):
            """forward 256-pt real DFT of window w -> bank w%2 of X0/X1 pair"""
            if w % 2 == 0:
                src, a = x_all, 3 * w // 2
            else:
                src, a = xsh_all, (3 * w - 1) // 2
            for b in range(2):
                for k in range(2):
                    nc.tensor.matmul(xh[b][:, w % 2, :], wf_t[:, b, k, :],
                                     src[:, a + k, :],
                                     start=(k == 0), stop=(k == 1))

        def pointwise(p, xh):
            """pair-fused elementwise: evacuate on ACT, multiply/add on DVE"""
            X0, X1 = xh
            x01 = ppool.tile([128, 2, 2, FREE], F16, tag="x01", name=f"x01_{p}")
            nc.scalar.copy(x01[:, 0, :, :], X0[:])
            nc.scalar.copy(x01[:, 1, :, :], X1[:])
            m0 = ppool.tile([128, 2, 2, FREE], F16, tag="m0", name=f"m0_{p}")
            nc.vector.tensor_mul(m0[:], x01[:], k_t[:, 0, :, :, :])
            y0 = ppool.tile([128, 2, FREE], F16, tag="y0", name=f"y0_{p}")
            nc.vector.tensor_add(y0[:], m0[:, 0, :, :], m0[:, 1, :, :])
            m1 = ppool.tile([128, 2, 2, FREE], F16, tag="m1", name=f"m1_{p}")
            nc.vector.tensor_mul(m1[:], x01[:], k_t[:, 1, :, :, :])
            y1 = ppool.tile([128, 2, FREE], F16, tag="y1", name=f"y1_{p}")
            nc.vector.tensor_add(y1[:], m1[:, 0, :, :], m1[:, 1, :, :])
            return y0, y1

        def inv_ranges(w):
            # (pair-slice, vseg, first_writer, window)
            if w % 2 == 0:
                return [(0, 0, True, w), (1, 1, True, w)]
            return [(1, 3, False, w), (2, 2, True, w)]

        def inv_pair(p, y0, y1):
            """inverse DFT for the pair into one 3-bank PSUM tile"""
            nb = min(3, NBLK - 3 * p)
            yi = ps_i.tile([128, 3, FREE], F32, tag="yi", name=f"yi{p}")
            ranges = inv_ranges(2 * p)
            if 2 * p + 1 < NW:
                ranges += inv_ranges(2 * p + 1)
            ranges = [r for r in ranges if r[0] < nb]
            for sl, seg, first, w in ranges:
                nc.tensor.matmul(yi[:, sl, :], vi_t[:, 0, seg, :],
                                 y0[:, w % 2, :], start=first, stop=False,
                                 skip_group_check=True)
            for sl, seg, first, w in ranges:
                nc.tensor.matmul(yi[:, sl, :], vi_t[:, 1, seg, :],
                                 y1[:, w % 2, :], start=False, stop=True,
                                 skip_group_check=True)
            return yi, nb

        def silu_store(p, yi, nb):
            """fused SiLU over the pair's 3 banks + one 384-row store.
            Emitted one pair LATE so it sits behind the next pair's ACT
            evacuation copies instead of blocking them."""
            o_sb = opool.tile([128, 3, FREE], F16, tag=f"o{p % 2}", name=f"o{p}")
            nc.scalar.activation(o_sb[:, 0:nb, :], yi[:, 0:nb, :],
                                 mybir.ActivationFunctionType.Silu)
            nc.sync.dma_start(
                out[384 * p: 384 * p + 128 * nb, :, :]
                .rearrange("(t p) b c -> p t (b c)", p=128),
                o_sb[:, 0:nb, :])

        # PE pre-warm: dummy matmuls keep the HAM activity monitor busy while
        # the first x tiles stream in, so real matmuls start near 2.4 GHz.
        warm = ps_i.tile([128, 3, FREE], F32, tag="yi", name="warm")
        for r in range(16):
            nc.tensor.matmul(warm[:, 0, 0:256], wf_t[:, 0, 0, :],
                             wf_t[:, 0, :, :], start=(r == 0), stop=(r == 15))
        # preload the sigmoid/silu ACT table set (~2.7 us) during the ramp so
        # the first real SiLU doesn't stall the ACT queue mid-pipeline.
        o_warm = opool.tile([128, 3, FREE], F16, tag="o0", name="o_warm")
        nc.scalar.activation(o_warm[:, 0, 0:8], warm[:, 0, 0:8],
                             mybir.ActivationFunctionType.Silu)

        def fwd_pair(p, xh):
            fwd(2 * p, xh)
            if 2 * p + 1 < NW:
                fwd(2 * p + 1, xh)

        def x_tiles(p):
            return [ps_f.tile([128, 2, FREE], F32, tag=f"x{b}",
                              name=f"x{b}_{p}") for b in range(2)]

        # pipeline: evacuate pair p (ACT), queue fwd(p+1) (waits only on the
        # evacuation), then pair p's DVE pointwise + inverse.
        xh_cur = x_tiles(0)
        fwd_pair(0, xh_cur)
        pend = None
        for p in range(NP):
            y0, y1 = pointwise(p, xh_cur)
            if pend is not None:
                silu_store(p - 1, *pend)
            if p + 1 < NP:
                xh_next = x_tiles(p + 1)
                fwd_pair(p + 1, xh_next)
            else:
                xh_next = None
            pend = inv_pair(p, y0, y1)
            xh_cur = xh_next
        silu_store(NP - 1, *pend)
    nc.compile()
    return nc


def _host_prep(x, alpha, delta, beta, gamma, omega):
    """Fold EMA params into freq-domain coefficient planes + DFT matrices."""
    a = 1.0 / (1.0 + np.exp(-alpha.astype(np.float64)))
    d = 1.0 / (1.0 + np.exp(-delta.astype(np.float64)))
    q = 1.0 - a * d                               # (2E, 16, 1)
    w = (a * beta.astype(np.float64))[:, :, 0] * gamma.astype(np.float64)
    w *= math.sqrt(1.0 / NDIM)                    # (2E, 16)
    tau = np.arange(128)
    kern = (w[:, :, None] * q[:, :, 0:1] ** tau[None, None, :]).sum(1)  # (2E,128)
    k1, k2 = kern[:E], kern[E:]
    kc = np.zeros((E, F))
    kc[:, 0:128] = k1
    kc[:, F - 127:] = k2[:, :127][:, ::-1]        # slot F-i holds k2[i-1]
    kc[:, 0] += omega.astype(np.float64)          # residual == omega on tap 0
    Khat = np.fft.rfft(kc, axis=1)                # (E, 129)
    KRe, KIm = Khat.real, Khat.imag

    planes = np.zeros((4, 128, E))                # A0, B0, A1, B1
    planes[0] = KRe[:, 0:128].T
    planes[1, 1:] = -KIm[:, 1:128].T
    planes[2, 0] = KRe[:, 128]
    planes[2, 1:] = KRe[:, 1:128].T
    planes[3, 1:] = KIm[:, 1:128].T

    # forward DFT lhsT packs: W0 (Re rows), W1 (Nyquist + Im rows) [256, 128]
    t_ = np.arange(F)
    fr = np.arange(128)
    W0 = np.cos(2 * np.pi * np.outer(t_, fr) / F)
    W1 = np.empty((F, 128))
    W1[:, 0] = np.cos(np.pi * t_)
    W1[:, 1:] = -np.sin(2 * np.pi * np.outer(t_, fr[1:]) / F)
    Wb = np.stack([W0, W1], axis=0)               # (2, 256, 128)
    wf = np.empty((128, 2, 2, 128))               # chunk k = rows 128k+p
    for k in range(2):
        wf[:, :, k, :] = Wb[:, 128 * k:128 * (k + 1), :].transpose(1, 0, 2)

    # inverse lhsT: V0/V1 [128 freq, 192 outs], zero-padded per range
    jj = np.arange(C) + T
    c_f = np.where(fr == 0, 1.0, 2.0)
    V0 = c_f[:, None] * np.cos(2 * np.pi * np.outer(fr, jj) / F) / F
    V1 = np.empty((128, C))
    V1[0] = ((-1.0) ** jj) / F
    V1[1:] = -2 * np.sin(2 * np.pi * np.outer(fr[1:], jj) / F) / F
    Vb = np.stack([V0, V1], axis=0)               # (2, 128, 192)
    vi = np.zeros((128, 2, 4, 128))
    vi[:, :, 0, :] = Vb[:, :, 0:128].transpose(1, 0, 2)      # even range A
    vi[:, :, 1, 0:64] = Vb[:, :, 128:192].transpose(1, 0, 2)  # even range B
    vi[:, :, 2, :] = Vb[:, :, 64:192].transpose(1, 0, 2)     # odd range B
    vi[:, :, 3, 64:128] = Vb[:, :, 0:64].transpose(1, 0, 2)  # odd range A

    xpad = np.zeros((NXT * 128, B, E), np.float16)
    xpad[T:T + L] = x.astype(np.float16)
    xsh = np.zeros((NXT * 128, B, E), np.float16)
    xsh[:NXT * 128 - 64] = xpad[64:]              # shifted copy for odd windows

    wf16 = np.ascontiguousarray(wf.astype(np.float16))
    vi16 = np.ascontiguousarray(vi.astype(np.float16))
    # fused-mul plane packs: m0 halves [A0 | B0], m1 halves [B1 | A1],
    # each repeated over the window pair axis.
    pk = np.stack([np.stack([planes[0], planes[1]]),
                   np.stack([planes[3], planes[2]])])   # (2, 2, 128, E)
    in_maps = []
    for core in range(N_CORES):
        sl = slice(core * ESH, (core + 1) * ESH)
        kc1 = np.broadcast_to(
            pk.reshape(2, 2, 128, 1, 1, E)[:, :, :, :, :, sl],
            (2, 2, 128, 2, B, ESH)).reshape(2, 2, 128, 2, FREE)
        kc1 = kc1.transpose(2, 0, 1, 3, 4)        # (128, m, half, pair, FREE)
        in_maps.append({
            "xs": np.ascontiguousarray(xpad[:, :, sl]),
            "xsh": np.ascontiguousarray(xsh[:, :, sl]),
            "wf": wf16,
            "vi": vi16,
            "kco": np.ascontiguousarray(kc1.astype(np.float16)),
        })
    return in_maps


def kernel(x, alpha, delta, beta, gamma, omega):
    global LAST_RESULTS
    if "nc" not in _CACHE:
        _CACHE["nc"] = _build_nc()
    nc = _CACHE["nc"]
    in_maps = _host_prep(x, alpha, delta, beta, gamma, omega)
    res = run_bass_kernel_spmd(nc, in_maps, core_ids=list(range(N_CORES)))
    LAST_RESULTS = res
    out = np.concatenate([res.results[c]["out"] for c in range(N_CORES)], axis=2)
    return out.astype(np.float32)


# revision 24
# speedup vs baseline: 1.3668x; 1.0038x over previous
"""MultiHeadEMA (MEGA bidirectional EMA + residual + SiLU) on 8 Trainium2 cores.

Strategy
--------
Per channel d (E=1024, B=4, L=4096):
    y[n] = silu( sum_{m<=n} x[m] k1[d, n-m] + sum_{m>n} x[m] k2[d, m-n-1]
                 + omega[d] x[n] )
with k1/k2 16-term geometric mixtures, q = 1 - sigmoid(a)*sigmoid(d) <= 0.865.
q^32 tail: worst-channel L1 1.4e-2 << 2e-2 * scale(16.9), so the length-2L FFT
conv reduces to a +-T=32-tap banded conv done by overlap-save with DFT F=256,
hop C=192 (22 windows).  E sharded 8 ways (128 ch/core, FREE = B*128 = 512).

Matmul cost on TRN2 is free-dim cycles (~216 ns at N=512 fp16) plus ~100 ns
whenever a partial-partition matmul breaks LDWEIGHTS pipelining, so every
matmul is full 128x128 config: x is staged TWICE (aligned + shifted 64 rows)
so odd windows contract 2 aligned tiles of the shifted copy, and half-block
inverse ranges use zero-padded V columns.  174 matmuls (88 fwd + 86 inv).

Freq packing (256-pt real DFT -> 256 real rows in 2 PSUM banks):
    X0 rows f=0..127:  Re X[f]
    X1 row 0: Re X[128] (Nyquist); rows 1..127: Im X[f]
Pointwise complex multiply, Y0 = A0*X0 + B0*X1 ; Y1 = A1*X1 + B1*X0, with
the omega residual folded into tap 0 host-side.

Engine law on TRN2 (measured): GpSimd tensor ops and DVE 2-port copy/cast
modes serialize on a shared SBUF port (exclusive lock), so GpSimd does NO
compute here and all PSUM evacuation runs on ScalarE.  Windows run in PAIRS:
    ACT : copy X0 pair + X1 pair PSUM -> one SBUF tile x01s (2x 1.15 us),
          one fused SiLU over the pair's 3 finished output banks (1.6 us)
    DVE : m0 = x01s*[A0|B0], m1 = x01s*[B1|A1] (FD=2048, 2x mode, 1.13 us
          each), y0/y1 = half + half adds (FD=1024, 0.6 us each)
    PE  : fwd 8 + inv 8 matmuls per pair (~220 ns each)
    GPS : only SWDGE issue of the shifted-x DMAs
Inverse matmuls are emitted y0-chunks-first; the pair's 3 output blocks
live in one 3-bank PSUM tile (split block accumulates both windows,
second window start=False), SiLU reads all 3 at once and one DMA stores
384 output rows.
"""

import math
import numpy as np
from contextlib import ExitStack

import concourse.bass as bass
import concourse.tile as tile
from concourse import bacc, mybir
from concourse.bass_utils import run_bass_kernel_spmd

L, B, E, NDIM = 4096, 4, 1024, 16
N_CORES = 8
ESH = E // N_CORES            # 128 channels per core
F, T, C = 256, 32, 192        # DFT length, one-sided taps, hop
NW = (L + C - 1) // C         # 22 windows (last covers 64 outputs)
NP = (NW + 1) // 2            # 11 window pairs
FREE = B * ESH                # 512 free elements (b, chan)
NXT = 33                      # x tiles: rows [0, 4224), x at [T, T+L)
NBLK = L // 128               # 32 output blocks

F16 = mybir.dt.float16
F32 = mybir.dt.float32

LAST_RESULTS = None           # BassKernelResults of the most recent run
_CACHE: dict = {}


def _build_nc():
    nc = bacc.Bacc("TRN2", target_bir_lowering=False, debug=False,
                   num_devices=N_CORES)
    xs = nc.dram_tensor("xs", [128, NXT, FREE], F16, kind="ExternalInput").ap()
    wf = nc.dram_tensor("wf", [128, 2, 2, 128], F16, kind="ExternalInput").ap()
    wo = nc.dram_tensor("wo", [128, 2, 3, 128], F16, kind="ExternalInput").ap()
    vi = nc.dram_tensor("vi", [128, 2, 4, 128], F16, kind="ExternalInput").ap()
    kco = nc.dram_tensor("kco", [128, 2, 2, 2, FREE], F16,
                         kind="ExternalInput").ap()
    out = nc.dram_tensor("out", [128, NBLK, FREE], F16, kind="ExternalOutput").ap()

    with ExitStack() as ctx:
        tc = ctx.enter_context(tile.TileContext(nc))
        cpool = ctx.enter_context(tc.tile_pool(name="const", bufs=1))
        ppool = ctx.enter_context(tc.tile_pool(name="pw", bufs=2))
        opool = ctx.enter_context(tc.tile_pool(name="outp", bufs=2))
        ps_f = ctx.enter_context(tc.tile_pool(name="psf", bufs=1, space="PSUM"))
        ps_i = ctx.enter_context(tc.tile_pool(name="psi", bufs=1, space="PSUM"))

        # DMA: consts + aligned x on Sync/Scalar HWDGE queues, shifted x via
        # GpSimd SWDGE (its only job).  Order = first-use order.
        x_all = cpool.tile([128, NXT, FREE], F16)
        nc.sync.dma_start(x_all[:, 0:4, :], xs[:, 0:4, :])
        wf_t = cpool.tile([128, 2, 2, 128], F16)
        nc.scalar.dma_start(wf_t[:], wf)
        wo_t = cpool.tile([128, 2, 3, 128], F16)
        nc.scalar.dma_start(wo_t[:], wo)
        k_t = cpool.tile([128, 2, 2, 2, FREE], F16)
        nc.scalar.dma_start(k_t[:], kco)
        vi_t = cpool.tile([128, 2, 4, 128], F16)
        nc.scalar.dma_start(vi_t[:], vi)
        for t0 in range(4, NXT, 6):
            t1 = min(t0 + 6, NXT)
            nc.sync.dma_start(x_all[:, t0:t1, :], xs[:, t0:t1, :])

        def fwd(w):
            """forward 256-pt real DFT of window w into its own 2-bank PSUM
            tile (double-buffered so the evacuation copy of window w runs
            while window w+1's matmuls stream).  Odd windows span 3 x tiles;
            edge chunks use zero-padded weight rows so every matmul stays
            full 128x128 config."""
            xw = ps_f.tile([128, 2, FREE], F32, tag="xw", name=f"xw_{w}",
                           bufs=2)
            if w % 2 == 0:
                a = 3 * w // 2
                chunks = [(a, wf_t, 0), (a + 1, wf_t, 1)]
            else:
                a = (3 * w - 1) // 2
                chunks = [(a, wo_t, 1), (a + 1, wo_t, 0)]
                if a + 2 < NXT:
                    chunks.append((a + 2, wo_t, 2))
            n = len(chunks)
            for b in range(2):
                for k, (t, wt, ki) in enumerate(chunks):
                    nc.tensor.matmul(xw[:, b, :], wt[:, b, ki, :],
                                     x_all[:, t, :],
                                     start=(k == 0), stop=(k == n - 1))
            return xw

        def pointwise(p, xh):
            """pair-fused elementwise: evacuate on ACT, multiply/add on DVE"""
            x01 = ppool.tile([128, 2, 2, FREE], F16, tag="x01", name=f"x01_{p}")
            for i, xw in enumerate(xh):
                nc.scalar.copy(x01[:, :, i, :], xw[:])
            m0 = ppool.tile([128, 2, 2, FREE], F16, tag="m0", name=f"m0_{p}")
            nc.vector.tensor_mul(m0[:], x01[:], k_t[:, 0, :, :, :])
            y0 = ppool.tile([128, 2, FREE], F16, tag="y0", name=f"y0_{p}")
            nc.vector.tensor_add(y0[:], m0[:, 0, :, :], m0[:, 1, :, :])
            m1 = ppool.tile([128, 2, 2, FREE], F16, tag="m1", name=f"m1_{p}")
            nc.vector.tensor_mul(m1[:], x01[:], k_t[:, 1, :, :, :])
            y1 = ppool.tile([128, 2, FREE], F16, tag="y1", name=f"y1_{p}")
            nc.vector.tensor_add(y1[:], m1[:, 0, :, :], m1[:, 1, :, :])
            return y0, y1

        def inv_ranges(w):
            # (pair-slice, vseg, first_writer, window)
            if w % 2 == 0:
                return [(0, 0, True, w), (1, 1, True, w)]
            return [(1, 3, False, w), (2, 2, True, w)]

        def inv_pair(p, y0, y1):
            """inverse DFT for the pair into one 3-bank PSUM tile"""
            nb = min(3, NBLK - 3 * p)
            yi = ps_i.tile([128, 3, FREE], F32, tag="yi", name=f"yi{p}")
            ranges = inv_ranges(2 * p)
            if 2 * p + 1 < NW:
                ranges += inv_ranges(2 * p + 1)
            ranges = [r for r in ranges if r[0] < nb]
            for sl, seg, first, w in ranges:
                nc.tensor.matmul(yi[:, sl, :], vi_t[:, 0, seg, :],
                                 y0[:, w % 2, :], start=first, stop=False,
                                 skip_group_check=True)
            for sl, seg, first, w in ranges:
                nc.tensor.matmul(yi[:, sl, :], vi_t[:, 1, seg, :],
                                 y1[:, w % 2, :], start=False, stop=True,
                                 skip_group_check=True)
            return yi, nb

        def silu_store(p, yi, nb):
            """fused SiLU over the pair's 3 banks + one 384-row store.
            Emitted one pair LATE so it sits behind the next pair's ACT
            evacuation copies instead of blocking them."""
            o_sb = opool.tile([128, 3, FREE], F16, tag=f"o{p % 2}", name=f"o{p}")
            nc.scalar.activation(o_sb[:, 0:nb, :], yi[:, 0:nb, :],
                                 mybir.ActivationFunctionType.Silu)
            nc.gpsimd.dma_start(out[:, 3 * p: 3 * p + nb, :], o_sb[:, 0:nb, :])

        # PE pre-warm: dummy matmuls keep the HAM activity monitor busy while
        # the first x tiles stream in, so real matmuls start near 2.4 GHz.
        warm = ps_i.tile([128, 3, FREE], F32, tag="yi", name="warm")
        garb = cpool.tile([128, 2, 128], F16)
        nc.vector.memset(garb[:], 0.25)
        for r in range(16):
            nc.tensor.matmul(warm[:, 0, 0:256], garb[:, 0, :],
                             garb[:, :, :], start=(r == 0), stop=(r == 15))
        # preload the sigmoid/silu ACT table set (~2.7 us) during the ramp so
        # the first real SiLU doesn't stall the ACT queue mid-pipeline.
        o_warm = opool.tile([128, 3, FREE], F16, tag="o0", name="o_warm")
        nc.scalar.activation(o_warm[:, 0, 0:8], warm[:, 0, 0:8],
                             mybir.ActivationFunctionType.Silu)

        def fwd_pair(p):
            xh = [fwd(2 * p)]
            if 2 * p + 1 < NW:
                xh.append(fwd(2 * p + 1))
            return xh

        # pipeline: evacuate pair p (ACT), queue fwd(p+1) (waits only on the
        # evacuation), then pair p's DVE pointwise + inverse.
        xh_cur = fwd_pair(0)
        pend = None
        for p in range(NP):
            y0, y1 = pointwise(p, xh_cur)
            if pend is not None:
                silu_store(p - 1, *pend)
            xh_cur = fwd_pair(p + 1) if p + 1 < NP else None
            pend = inv_pair(p, y0, y1)
        silu_store(NP - 1, *pend)
    nc.compile()
    return nc


def _host_prep(x, alpha, delta, beta, gamma, omega):
    """Fold EMA params into freq-domain coefficient planes + DFT matrices."""
    a = 1.0 / (1.0 + np.exp(-alpha.astype(np.float64)))
    d = 1.0 / (1.0 + np.exp(-delta.astype(np.float64)))
    q = 1.0 - a * d                               # (2E, 16, 1)
    w = (a * beta.astype(np.float64))[:, :, 0] * gamma.astype(np.float64)
    w *= math.sqrt(1.0 / NDIM)                    # (2E, 16)
    tau = np.arange(128)
    kern = (w[:, :, None] * q[:, :, 0:1] ** tau[None, None, :]).sum(1)  # (2E,128)
    k1, k2 = kern[:E], kern[E:]
    kc = np.zeros((E, F))
    kc[:, 0:128] = k1
    kc[:, F - 127:] = k2[:, :127][:, ::-1]        # slot F-i holds k2[i-1]
    kc[:, 0] += omega.astype(np.float64)          # residual == omega on tap 0
    Khat = np.fft.rfft(kc, axis=1)                # (E, 129)
    KRe, KIm = Khat.real, Khat.imag

    planes = np.zeros((4, 128, E))                # A0, B0, A1, B1
    planes[0] = KRe[:, 0:128].T
    planes[1, 1:] = -KIm[:, 1:128].T
    planes[2, 0] = KRe[:, 128]
    planes[2, 1:] = KRe[:, 1:128].T
    planes[3, 1:] = KIm[:, 1:128].T

    # forward DFT lhsT packs: W0 (Re rows), W1 (Nyquist + Im rows) [256, 128]
    t_ = np.arange(F)
    fr = np.arange(128)
    W0 = np.cos(2 * np.pi * np.outer(t_, fr) / F)
    W1 = np.empty((F, 128))
    W1[:, 0] = np.cos(np.pi * t_)
    W1[:, 1:] = -np.sin(2 * np.pi * np.outer(t_, fr[1:]) / F)
    Wb = np.stack([W0, W1], axis=0)               # (2, 256, 128)
    wf = np.empty((128, 2, 2, 128))               # chunk k = rows 128k+p
    for k in range(2):
        wf[:, :, k, :] = Wb[:, 128 * k:128 * (k + 1), :].transpose(1, 0, 2)
    # odd windows: full 128-part chunks with zero-padded edge weights
    wo = np.zeros((128, 2, 3, 128))
    wo[:, :, 0, :] = Wb[:, 64:192, :].transpose(1, 0, 2)      # mid: tile a+1
    wo[64:128, :, 1, :] = Wb[:, 0:64, :].transpose(1, 0, 2)   # lo: tile a
    wo[0:64, :, 2, :] = Wb[:, 192:256, :].transpose(1, 0, 2)  # hi: tile a+2

    # inverse lhsT: V0/V1 [128 freq, 192 outs], zero-padded per range
    jj = np.arange(C) + T
    c_f = np.where(fr == 0, 1.0, 2.0)
    V0 = c_f[:, None] * np.cos(2 * np.pi * np.outer(fr, jj) / F) / F
    V1 = np.empty((128, C))
    V1[0] = ((-1.0) ** jj) / F
    V1[1:] = -2 * np.sin(2 * np.pi * np.outer(fr[1:], jj) / F) / F
    Vb = np.stack([V0, V1], axis=0)               # (2, 128, 192)
    vi = np.zeros((128, 2, 4, 128))
    vi[:, :, 0, :] = Vb[:, :, 0:128].transpose(1, 0, 2)      # even range A
    vi[:, :, 1, 0:64] = Vb[:, :, 128:192].transpose(1, 0, 2)  # even range B
    vi[:, :, 2, :] = Vb[:, :, 64:192].transpose(1, 0, 2)     # odd range B
    vi[:, :, 3, 64:128] = Vb[:, :, 0:64].transpose(1, 0, 2)  # odd range A

    xpad = np.zeros((NXT * 128, B, E), np.float16)
    xpad[T:T + L] = x.astype(np.float16)
    # partition-major staging: [128, NXT, B, E] so DMA partition lines are
    # long and contiguous (strided 1KB lines measured ~5x slower).
    xpad = np.ascontiguousarray(xpad.reshape(NXT, 128, B, E).transpose(1, 0, 2, 3))

    wf16 = np.ascontiguousarray(wf.astype(np.float16))
    wo16 = np.ascontiguousarray(wo.astype(np.float16))
    vi16 = np.ascontiguousarray(vi.astype(np.float16))
    # fused-mul plane packs: m0 halves [A0 | B0], m1 halves [B1 | A1],
    # each repeated over the window pair axis.
    pk = np.stack([np.stack([planes[0], planes[1]]),
                   np.stack([planes[3], planes[2]])])   # (2, 2, 128, E)
    in_maps = []
    for core in range(N_CORES):
        sl = slice(core * ESH, (core + 1) * ESH)
        kc1 = np.broadcast_to(
            pk.reshape(2, 2, 128, 1, 1, E)[:, :, :, :, :, sl],
            (2, 2, 128, 2, B, ESH)).reshape(2, 2, 128, 2, FREE)
        kc1 = kc1.transpose(2, 0, 1, 3, 4)        # (128, m, half, pair, FREE)
        in_maps.append({
            "xs": np.ascontiguousarray(xpad[:, :, :, sl]).reshape(128, NXT, FREE),
            "wf": wf16,
            "wo": wo16,
            "vi": vi16,
            "kco": np.ascontiguousarray(kc1.astype(np.float16)),
        })
    return in_maps


def kernel(x, alpha, delta, beta, gamma, omega):
    global LAST_RESULTS
    if "nc" not in _CACHE:
        _CACHE["nc"] = _build_nc()
    nc = _CACHE["nc"]
    in_maps = _host_prep(x, alpha, delta, beta, gamma, omega)
    res = run_bass_kernel_spmd(nc, in_maps, core_ids=list(range(N_CORES)))
    LAST_RESULTS = res
    outs = []
    for c in range(N_CORES):
        o = res.results[c]["out"].reshape(128, NBLK, B, ESH)
        outs.append(o.transpose(1, 0, 2, 3).reshape(L, B, ESH))
    return np.concatenate(outs, axis=2).astype(np.float32)
